# revision 39
# baseline (speedup 1.0000x reference)
"""Multi-head self-attention Trainium2 kernel (8-core head-parallel, v3).

Problem: B=2, N=2048, C=1024, H=16 heads, HD=64.

The graded wall-time is dominated by host<->device I/O shipping (the axon
tunnel moves every NEFF ExternalInput/Output on each call at ~0.5 ms/MB/
core), so this version minimizes per-call tunnel bytes:

  * weights are baked into the NEFF as inline consts (loaded to device
    DRAM once at model-load time, not per call); each core picks its
    per-head slice with partition_id-predicated DMAs.
  * input: each core receives only ITS token shard of x (x^T slice
    [C, 512] in fp16, 1 MB); the full x^T is reassembled on-device with
    an AllGather over NeuronLink.
  * output: the 8 partial output projections are summed on-device with
    per-batch ReduceScatter(add) (batch 0's reduce overlaps batch 1's
    compute), so each core ships back only 512 rows of the final
    [4096, 1024] output in fp16 (1 MB).

Compute (per core, 2 heads): all matmuls in fp16 (double PE rate):
  1. qkv: q^T,k^T = w^T @ x^T per 512-token block (contraction over C in
     8 chunks), q/k bias added during PSUM->SBUF evacuation.  v is
     produced directly in natural [token, feat] layout (x-chunk
     stationary, w_v moving) so no PE transpose is needed; a constant
     1.0 column per head is memset so attn@v also yields the softmax
     denominators.  The v bias is folded into the host-side output bias
     (softmax rows sum to 1, so it passes through as b_v @ w_proj).
  2. attention per (batch, head): score chunks on PE, exp(s/8 - 4) on
     ACT straight out of PSUM (the -4 bias cancels in normalization and
     keeps fp16 exp comfortably in range), attn@v accumulated over 16
     key chunks into PSUM [65, 512] (row 64 = denominators).
  3. normalization: denominators broadcast across partitions via a PE
     outer product, reciprocal + multiply on DVE -> oh^T fp16.
  4. partial projection oh^T @ w2 -> DRAM bounce, then ReduceScatter.
"""

import numpy as np

B, N, C = 2, 2048, 1024
H = 16
HD = C // H  # 64
SCALE = HD ** -0.5
T = B * N  # 4096 tokens
NCORES = 8
HPC = H // NCORES  # 2 heads per core
SHARD = T // NCORES  # 512 tokens per core
OSH = N // NCORES  # 256 output rows per core per batch
EXP_BIAS = -4.0

_CACHE = {}


def _prep_weights(w_qkv, b_qkv, w_proj):
    """Stack per-core weight slices for the inline-const tensors."""
    wq_all = np.empty((NCORES * C, 3 * HPC * HD), np.float16)
    b_all = np.empty((NCORES * 128, 2), np.float32)
    w2_all = np.empty((NCORES * HPC * HD, C), np.float16)
    for core in range(NCORES):
        heads = [core * HPC + h for h in range(HPC)]
        cols = []
        for s in range(3):  # q, k, v groups -> [qA qB kA kB vA vB]
            for h in heads:
                cols.append(np.arange(s * C + h * HD, s * C + (h + 1) * HD))
        cols = np.concatenate(cols)
        wq_all[core * C:(core + 1) * C] = w_qkv[:, cols].astype(np.float16)
        b_all[core * 128:(core + 1) * 128] = (
            b_qkv[cols[:256]].reshape(2, HPC * HD).T.astype(np.float32))
        rows = np.concatenate(
            [np.arange(h * HD, (h + 1) * HD) for h in heads])
        w2_all[core * 128:(core + 1) * 128] = w_proj[rows, :].astype(np.float16)
    return wq_all, b_all, w2_all


def _build_program(w_qkv, b_qkv, w_proj, reps=1, sim_mode=False,
                   ag_split=True, rs_split=True):
    # sim_mode: skip collectives (unsupported by TimelineSim) so the compute
    # portion can be timeline-profiled single-core; numerics are garbage.
    # ag_split/rs_split: emit the x AllGather / output ReduceScatter as two
    # halves (overlap) or one collective each (less per-collective overhead).
    import concourse.bass as bass
    import concourse.mybir as mybir
    import concourse.tile as tile
    from concourse import bacc

    f16 = mybir.dt.float16
    f32 = mybir.dt.float32
    Exp = mybir.ActivationFunctionType.Exp
    Mult = mybir.AluOpType.mult

    wq_all, b_all, w2_all = _prep_weights(w_qkv, b_qkv, w_proj)

    nc = bacc.Bacc("TRN2", target_bir_lowering=False, debug=False,
                   num_devices=NCORES)

    xs_d = nc.dram_tensor("xs", [C, SHARD], f16, kind="ExternalInput")
    out_d = nc.dram_tensor("out_sh", [B, OSH, C], f16, kind="ExternalOutput")

    wq_c = nc.inline_tensor(wq_all, "wq_c")
    b_c = nc.inline_tensor(b_all, "b_c")
    w2_c = nc.inline_tensor(w2_all, "w2_c")

    # collective bounce buffers (outputs Shared for the fast HBM-HBM path).
    # The x AllGather is split by channel halves so the qkv contraction can
    # start on ci 0..3 while the second half is still gathering.
    NAG = 2 if ag_split else 1
    CH = C // NAG
    xg_in_h = [nc.dram_tensor(f"xg_in{h}", [CH, SHARD], f16,
                              kind="Internal") for h in range(NAG)]
    xg_h = [nc.dram_tensor(f"xg{h}", [NCORES * CH, SHARD], f16,
                           kind="Internal", addr_space="Shared")
            for h in range(NAG)]
    NRS = B if rs_split else 1
    RSROWS = T // NRS
    op_b = [nc.dram_tensor(f"op{b}", [RSROWS, C], f16, kind="Internal")
            for b in range(NRS)]
    os_b = [nc.dram_tensor(f"os{b}", [RSROWS // NCORES, C], f16,
                           kind="Internal") for b in range(NRS)]

    CC = C // 128          # 8 contraction chunks
    NTB = T // 512         # 8 token blocks (= shards)
    NKC = N // 128         # 16 key chunks per batch
    NQB = N // 512         # 4 query blocks per batch
    NTC = T // 128         # 32 token chunks
    GROUPS = [list(range(NCORES))]

    pid = nc.partition_id()

    with tile.TileContext(nc) as tc:
        with tc.tile_pool(name="persist", bufs=1) as persist, \
             tc.tile_pool(name="xt", bufs=3, space="SBUF") as xt_pool, \
             tc.tile_pool(name="exp", bufs=6) as exp_pool, \
             tc.tile_pool(name="small", bufs=4) as small_pool, \
             tc.tile_pool(name="ob", bufs=3) as out_pool, \
             tc.tile_pool(name="ps", bufs=2, space="PSUM") as psum_s, \
             tc.tile_pool(name="aux", bufs=1, space="PSUM") as psum_aux, \
             tc.tile_pool(name="po", bufs=2, space="PSUM") as psum_o:

            w_sb = persist.tile([128, CC, 3 * HPC * HD], f16, tag="w_sb")
            b_sb = persist.tile([128, 2], f32, tag="b_sb")
            w2_sb = persist.tile([128, C], f16, tag="w2_sb")
            qT = persist.tile([128, T], f16, tag="qT")
            kT = persist.tile([128, T], f16, tag="kT")
            # natural-layout v, per token-chunk: [vA(64) | 1 | vB(64) | 1]
            v_nat = persist.tile([128, NTC, 130], f16, tag="v_nat")
            ohT = persist.tile([128, T], f16, tag="ohT")
            ones64 = persist.tile([1, 64], f16, tag="ones64")
            bias_m4 = persist.tile([128, 1], f32, tag="bias_m4")

            # per-core weight selection: 8 predicated DMAs, 7 skip
            for c in range(NCORES):
                cond = pid == c
                nc.sync.dma_start(
                    out=w_sb[:],
                    in_=wq_c[c * C:(c + 1) * C, :].rearrange(
                        "(cc p) f -> p cc f", p=128),
                    cond=cond)
                nc.sync.dma_start(
                    out=b_sb[:], in_=b_c[c * 128:(c + 1) * 128, :], cond=cond)
                nc.sync.dma_start(
                    out=w2_sb[:], in_=w2_c[c * 128:(c + 1) * 128, :], cond=cond)
            nc.vector.memset(ones64[:], 1.0)
            nc.vector.memset(bias_m4[:], EXP_BIAS)

            def emit_body(rep):
                # constant 1.0 columns (per-head softmax-denominator rows)
                nc.vector.memset(v_nat[:, :, 64:65], 1.0)
                nc.vector.memset(v_nat[:, :, 129:130], 1.0)

                for h in range(NAG):
                    nc.scalar.dma_start(
                        out=xg_in_h[h][:],
                        in_=xs_d[h * CH:(h + 1) * CH, :])
                for h in range(NAG):
                    if not sim_mode:
                        nc.gpsimd.collective_compute(
                            "AllGather", mybir.AluOpType.bypass,
                            replica_groups=GROUPS,
                            ins=[xg_in_h[h][:].opt()],
                            outs=[xg_h[h][:].opt()])

                # ---- phase 1 (per batch): q^T,k^T = w^T @ x^T with bias on
                # evac; v computed in natural [token, feat] layout
                def emit_qkv(tb):
                    xt = xt_pool.tile([128, CC, 512], f16, tag="xt",
                                      name=f"xt_{rep}_{tb}")
                    for h in range(NAG):
                        nc.sync.dma_start(
                            out=xt[:, h * (CC // NAG):(h + 1) * (CC // NAG), :],
                            in_=xg_h[h][tb * CH:(tb + 1) * CH, :].rearrange(
                                "(cc p) t -> p cc t", p=128))
                    xts = [xt[:, ci, :] for ci in range(CC)]
                    for fc in range(2):
                        ps = psum_s.tile([128, 512], f32, tag="s",
                                         name=f"ps1_{rep}_{tb}_{fc}")
                        for ci in range(CC):
                            nc.tensor.matmul(
                                ps[:],
                                w_sb[:, ci, fc * 128:(fc + 1) * 128],
                                xts[ci],
                                start=(ci == 0), stop=(ci == CC - 1))
                        nc.vector.tensor_scalar_add(
                            (qT if fc == 0 else kT)[:, tb * 512:(tb + 1) * 512],
                            ps[:], b_sb[:, fc:fc + 1])
                    for tcq in range(4):
                        tcg = tb * 4 + tcq
                        pv = psum_o.tile([128, 512], f32, tag="po",
                                         name=f"pv_{rep}_{tcg}")
                        for ci in range(CC):
                            nc.tensor.matmul(
                                pv[:, 0:128],
                                xt[:, ci, tcq * 128:(tcq + 1) * 128],
                                w_sb[:, ci, 256:384],
                                start=(ci == 0), stop=(ci == CC - 1))
                        # strided copy: pv cols [0:64],[64:128] land at
                        # v_nat[:, tcg, 0:64] and [65:129] (skip ones col)
                        src = pv[:, 0:128]
                        dst = v_nat[:, tcg, 0:129]
                        nc.vector.tensor_copy(
                            bass.AP(tensor=dst.tensor, offset=dst.offset,
                                    ap=[list(dst.ap[0]), [65, 2], [1, 64]]),
                            bass.AP(tensor=src.tensor, offset=src.offset,
                                    ap=[list(src.ap[0]), [64, 2], [1, 64]]))

                # ---- phase 2: attention per (batch, head) ----
                def emit_attention(b):
                    for qb in range(NQB):
                        qsl = slice(b * N + qb * 512, b * N + (qb + 1) * 512)
                        po = [psum_o.tile([128, 512], f32, tag="po",
                                          name=f"po_{rep}_{b}_{qb}_{h}")
                              for h in range(HPC)]
                        for kcg in range(NKC // 2):
                            exs = {}
                            for h in range(HPC):
                                hsl = slice(h * 64, (h + 1) * 64)
                                ps = psum_s.tile(
                                    [128, 1024], f32, tag="s",
                                    name=f"ps2_{rep}_{b}_{qb}_{kcg}_{h}")
                                for kc2 in range(2):
                                    kc = kcg * 2 + kc2
                                    ksl = slice(b * N + kc * 128,
                                                b * N + (kc + 1) * 128)
                                    nc.tensor.matmul(
                                        ps[:, kc2 * 512:(kc2 + 1) * 512],
                                        kT[hsl, ksl], qT[hsl, qsl],
                                        start=True, stop=True)
                                ex = exp_pool.tile(
                                    [128, 1024], f16, tag="ex",
                                    name=f"ex_{rep}_{b}_{qb}_{kcg}_{h}")
                                nc.scalar.activation(ex[:], ps[:], Exp,
                                                     scale=float(SCALE),
                                                     bias=bias_m4[:])
                                exs[h] = ex
                            for kc2 in range(2):
                                kc = kcg * 2 + kc2
                                tcg = b * NKC + kc
                                for h in range(HPC):
                                    nc.tensor.matmul(
                                        po[h][0:65, :],
                                        v_nat[:, tcg, h * 65:(h + 1) * 65],
                                        exs[h][:, kc2 * 512:(kc2 + 1) * 512],
                                        start=(kc == 0),
                                        stop=(kc == NKC - 1))
                        for h in range(HPC):
                            # broadcast denom row across partitions via a PE
                            # outer product, then reciprocal + multiply on DVE
                            s_sb = small_pool.tile(
                                [1, 512], f16, tag="r",
                                name=f"s_sb_{rep}_{b}_{qb}_{h}")
                            nc.vector.tensor_copy(s_sb[:], po[h][64:65, :])
                            pr = psum_aux.tile([64, 512], f32, tag="aux",
                                               name=f"pr_{rep}_{b}_{qb}_{h}")
                            nc.tensor.matmul(pr[:], ones64[:], s_sb[:],
                                             start=True, stop=True)
                            rcp = small_pool.tile(
                                [64, 512], f32, tag="rb",
                                name=f"rcp_{rep}_{b}_{qb}_{h}")
                            nc.vector.reciprocal(rcp[:], pr[:])
                            nc.vector.tensor_tensor(
                                ohT[h * 64:(h + 1) * 64, qsl],
                                po[h][0:64, :], rcp[:], Mult)

                        # ---- phase 3 interleaved: project this q-block's
                        # 4 token chunks while the next q-block computes ----
                        for tcq in range(4):
                            tcg = b * 16 + qb * 4 + tcq
                            pp = psum_aux.tile([128, 1024], f32, tag="aux",
                                               name=f"pp_{rep}_{tcg}")
                            for jh in range(C // 512):
                                nc.tensor.matmul(
                                    pp[:, jh * 512:(jh + 1) * 512],
                                    ohT[:, tcg * 128:(tcg + 1) * 128],
                                    w2_sb[:, jh * 512:(jh + 1) * 512],
                                    start=True, stop=True)
                            ob = out_pool.tile([128, 1024], f16, tag="ob",
                                               name=f"ob_{rep}_{tcg}")
                            nc.vector.tensor_copy(ob[:], pp[:])
                            rsb = b if rs_split else 0
                            lr = tcg * 128 - rsb * N  # row within op_b[rsb]
                            nc.sync.dma_start(
                                out=op_b[rsb][lr:lr + 128, :], in_=ob[:])

                for b in range(B):
                    for tb in range(b * NTB // B, (b + 1) * NTB // B):
                        emit_qkv(tb)
                    emit_attention(b)
                    # batch b's reduce-scatter overlaps batch b+1's compute
                    if rs_split:
                        if not sim_mode:
                            nc.gpsimd.collective_compute(
                                "ReduceScatter", mybir.AluOpType.add,
                                replica_groups=GROUPS,
                                ins=[op_b[b][:].opt()],
                                outs=[os_b[b][:].opt()])
                        nc.sync.dma_start(out=out_d[b, :, :], in_=os_b[b][:])
                if not rs_split:
                    if not sim_mode:
                        nc.gpsimd.collective_compute(
                            "ReduceScatter", mybir.AluOpType.add,
                            replica_groups=GROUPS,
                            ins=[op_b[0][:].opt()], outs=[os_b[0][:].opt()])
                    nc.sync.dma_start(out=out_d[:].opt(),
                                      in_=os_b[0][:].opt())

            for rep in range(reps):
                emit_body(rep)

    nc.compile()
    return nc


def _weights_key(w_qkv, b_qkv, w_proj):
    import hashlib
    h = hashlib.sha1()
    for a in (w_qkv, b_qkv, w_proj):
        h.update(np.ascontiguousarray(a, dtype=np.float32).tobytes())
    return h.hexdigest()


def get_program(w_qkv, b_qkv, w_proj):
    key = _weights_key(w_qkv, b_qkv, w_proj)
    if _CACHE.get("key") != key:
        _CACHE["nc"] = _build_program(w_qkv, b_qkv, w_proj)
        _CACHE["key"] = key
    return _CACHE["nc"]


def build_null_program():
    """Tiny kernel for calibrating per-dispatch overhead in test harnesses."""
    import concourse.mybir as mybir
    import concourse.tile as tile
    from concourse import bacc

    f32 = mybir.dt.float32
    nc = bacc.Bacc("TRN2", target_bir_lowering=False, debug=False,
                   num_devices=NCORES)
    x_in = nc.dram_tensor("x", [128, 128], f32, kind="ExternalInput")
    y_out = nc.dram_tensor("y", [128, 128], f32, kind="ExternalOutput")
    with tile.TileContext(nc) as tc:
        with tc.tile_pool(name="p", bufs=1) as pool:
            t = pool.tile([128, 128], f32)
            nc.sync.dma_start(out=t[:], in_=x_in[:])
            nc.sync.dma_start(out=y_out[:], in_=t[:])
    nc.compile()
    x = np.zeros((128, 128), dtype=np.float32)
    return nc, [{"x": x} for _ in range(NCORES)]


def make_in_maps(x, w_qkv=None, b_qkv=None, w_proj=None):
    """Host-side sharding: per-core input dicts (fp16 x-shard only)."""
    xT = np.ascontiguousarray(x.reshape(T, C).T).astype(np.float16)
    return [{"xs": np.ascontiguousarray(
        xT[:, core * SHARD:(core + 1) * SHARD])} for core in range(NCORES)]


def combine_results(results, b_qkv, w_proj, b_proj):
    """Host-side unshard: interleave the per-batch output shards, add the
    effective bias (v bias passes through softmax + projection)."""
    b_eff = (b_proj.astype(np.float64)
             + b_qkv[2 * C:].astype(np.float64) @ w_proj.astype(np.float64))
    acc = np.empty((B, N, C), np.float32)
    for c, res in enumerate(results):
        sh = np.asarray(res["out_sh"]).astype(np.float32)
        for b in range(B):
            acc[b, c * OSH:(c + 1) * OSH] = sh[b]
    return acc + b_eff.astype(np.float32)[None, None, :]


def kernel(x, w_qkv, b_qkv, w_proj, b_proj):
    from concourse.bass_utils import run_bass_kernel_spmd

    x = np.asarray(x, dtype=np.float32)
    w_qkv = np.asarray(w_qkv, dtype=np.float32)
    b_qkv = np.asarray(b_qkv, dtype=np.float32)
    w_proj = np.asarray(w_proj, dtype=np.float32)
    b_proj = np.asarray(b_proj, dtype=np.float32)

    nc = get_program(w_qkv, b_qkv, w_proj)
    in_maps = make_in_maps(x)
    res = run_bass_kernel_spmd(nc, in_maps, list(range(NCORES)))
    return combine_results(res.results, b_qkv, w_proj, b_proj)


# revision 46
# speedup vs baseline: 4.5936x; 4.5936x over previous
"""Multi-head self-attention Trainium2 kernel (8-core head-parallel, v3).

Problem: B=2, N=2048, C=1024, H=16 heads, HD=64.

The graded wall-time is dominated by host<->device I/O shipping (the axon
tunnel moves every NEFF ExternalInput/Output on each call at ~0.5 ms/MB/
core), so this version minimizes per-call tunnel bytes:

  * weights are baked into the NEFF as inline consts (loaded to device
    DRAM once at model-load time, not per call); each core picks its
    per-head slice with partition_id-predicated DMAs.
  * input: each core receives only ITS token shard of x (x^T slice
    [C, 512] in fp16, 1 MB); the full x^T is reassembled on-device with
    an AllGather over NeuronLink.
  * output: the 8 partial output projections are summed on-device with
    per-batch ReduceScatter(add) (batch 0's reduce overlaps batch 1's
    compute), so each core ships back only 512 rows of the final
    [4096, 1024] output in fp16 (1 MB).

Compute (per core, 2 heads): all matmuls in fp16 (double PE rate):
  1. qkv: q^T,k^T = w^T @ x^T per 512-token block (contraction over C in
     8 chunks), q/k bias added during PSUM->SBUF evacuation.  v is
     produced directly in natural [token, feat] layout (x-chunk
     stationary, w_v moving) so no PE transpose is needed; a constant
     1.0 column per head is memset so attn@v also yields the softmax
     denominators.  The v bias is folded into the host-side output bias
     (softmax rows sum to 1, so it passes through as b_v @ w_proj).
  2. attention per (batch, head): score chunks on PE, exp(s/8 - 4) on
     ACT straight out of PSUM (the -4 bias cancels in normalization and
     keeps fp16 exp comfortably in range), attn@v accumulated over 16
     key chunks into PSUM [65, 512] (row 64 = denominators).
  3. normalization: denominators broadcast across partitions via a PE
     outer product, reciprocal + multiply on DVE -> oh^T fp16.
  4. partial projection oh^T @ w2 -> DRAM bounce, then ReduceScatter.
"""

import numpy as np

B, N, C = 2, 2048, 1024
H = 16
HD = C // H  # 64
SCALE = HD ** -0.5
T = B * N  # 4096 tokens
NCORES = 8
HPC = H // NCORES  # 2 heads per core
SHARD = T // NCORES  # 512 tokens per core
OSH = N // NCORES  # 256 output rows per core per batch
EXP_BIAS = -4.0

_CACHE = {}


def _prep_weights(w_qkv, b_qkv, w_proj):
    """Stack per-core weight slices for the inline-const tensors.

    wq is stored partition-major ([128, CC*384] per core) so the SBUF load
    is one fully-contiguous DMA instead of 1024 x 768B strided rows."""
    CC = C // 128
    F = 3 * HPC * HD
    wq_all = np.empty((NCORES * 128, CC * F), np.float16)
    b_all = np.empty((NCORES * 128, 2), np.float32)
    w2_all = np.empty((NCORES * HPC * HD, C), np.float16)
    for core in range(NCORES):
        heads = [core * HPC + h for h in range(HPC)]
        cols = []
        for s in range(3):  # q, k, v groups -> [qA qB kA kB vA vB]
            for h in heads:
                cols.append(np.arange(s * C + h * HD, s * C + (h + 1) * HD))
        cols = np.concatenate(cols)
        wq_core = w_qkv[:, cols].astype(np.float16)  # [C, F]
        # [C, F] -> [p, cc, F] -> [128, CC*F]
        wq_all[core * 128:(core + 1) * 128] = (
            wq_core.reshape(CC, 128, F).transpose(1, 0, 2).reshape(128, CC * F))
        b_all[core * 128:(core + 1) * 128] = (
            b_qkv[cols[:256]].reshape(2, HPC * HD).T.astype(np.float32))
        rows = np.concatenate(
            [np.arange(h * HD, (h + 1) * HD) for h in heads])
        w2_all[core * 128:(core + 1) * 128] = w_proj[rows, :].astype(np.float16)
    return wq_all, b_all, w2_all


def _build_program(w_qkv, b_qkv, w_proj, reps=1, sim_mode=False,
                   ag_split=True, rs_split=True):
    # sim_mode: skip collectives (unsupported by TimelineSim) so the compute
    # portion can be timeline-profiled single-core; numerics are garbage.
    # ag_split/rs_split: emit the x AllGather / output ReduceScatter as two
    # halves (overlap) or one collective each (less per-collective overhead).
    import concourse.bass as bass
    import concourse.mybir as mybir
    import concourse.tile as tile
    from concourse import bacc

    f16 = mybir.dt.float16
    f32 = mybir.dt.float32
    Exp = mybir.ActivationFunctionType.Exp
    Mult = mybir.AluOpType.mult

    wq_all, b_all, w2_all = _prep_weights(w_qkv, b_qkv, w_proj)

    nc = bacc.Bacc("TRN2", target_bir_lowering=False, debug=False,
                   num_devices=NCORES)

    # x shard ships partition-major per channel-half ([2*128, 4*512]) so the
    # bounce, the gathered-block reads, and the SBUF tile are all contiguous
    CC = C // 128
    HCC = CC // 2
    xs_d = nc.dram_tensor("xs", [2 * 128, HCC * SHARD], f16,
                          kind="ExternalInput")
    out_d = nc.dram_tensor("out_sh", [B, OSH, C], f16, kind="ExternalOutput")

    wq_c = nc.inline_tensor(wq_all, "wq_c")
    b_c = nc.inline_tensor(b_all, "b_c")
    w2_c = nc.inline_tensor(w2_all, "w2_c")

    # collective bounce buffers (outputs Shared for the fast HBM-HBM path).
    # The x AllGather is split by channel halves so the qkv contraction can
    # start on ci 0..3 while the second half is still gathering.
    NAG = 2 if ag_split else 1
    HROWS = 256 // NAG  # rows per gather chunk (128 per half, or all 256)
    xg_in_h = [nc.dram_tensor(f"xg_in{h}", [HROWS, HCC * SHARD], f16,
                              kind="Internal") for h in range(NAG)]
    xg_h = [nc.dram_tensor(f"xg{h}", [NCORES * HROWS, HCC * SHARD], f16,
                           kind="Internal", addr_space="Shared")
            for h in range(NAG)]
    NRS = B if rs_split else 1
    RSROWS = T // NRS
    op_b = [nc.dram_tensor(f"op{b}", [RSROWS, C], f16, kind="Internal")
            for b in range(NRS)]
    os_b = [nc.dram_tensor(f"os{b}", [RSROWS // NCORES, C], f16,
                           kind="Internal") for b in range(NRS)]

    CC = C // 128          # 8 contraction chunks
    NTB = T // 512         # 8 token blocks (= shards)
    NKC = N // 128         # 16 key chunks per batch
    NQB = N // 512         # 4 query blocks per batch
    NTC = T // 128         # 32 token chunks
    GROUPS = [list(range(NCORES))]

    pid = nc.partition_id()

    with tile.TileContext(nc) as tc:
        with tc.tile_pool(name="persist", bufs=1) as persist, \
             tc.tile_pool(name="xt", bufs=3, space="SBUF") as xt_pool, \
             tc.tile_pool(name="exp", bufs=6) as exp_pool, \
             tc.tile_pool(name="small", bufs=4) as small_pool, \
             tc.tile_pool(name="ob", bufs=3) as out_pool, \
             tc.tile_pool(name="ps", bufs=2, space="PSUM") as psum_s, \
             tc.tile_pool(name="aux", bufs=1, space="PSUM") as psum_aux, \
             tc.tile_pool(name="po", bufs=2, space="PSUM") as psum_o:

            w_sb = persist.tile([128, CC, 3 * HPC * HD], f16, tag="w_sb")
            b_sb = persist.tile([128, 2], f32, tag="b_sb")
            w2_sb = persist.tile([128, C], f16, tag="w2_sb")
            qT = persist.tile([128, T], f16, tag="qT")
            kT = persist.tile([128, T], f16, tag="kT")
            # natural-layout v, per token-chunk: [vA(64) | 1 | vB(64) | 1]
            v_nat = persist.tile([128, NTC, 130], f16, tag="v_nat")
            ohT = persist.tile([128, T], f16, tag="ohT")
            ones64 = persist.tile([1, 64], f16, tag="ones64")
            bias_m4 = persist.tile([128, 1], f32, tag="bias_m4")

            # per-core weight selection: 8 predicated DMAs, 7 skip
            # (wq const is partition-major: contiguous [128, CC*384] rows)
            for c in range(NCORES):
                cond = pid == c
                nc.sync.dma_start(
                    out=w_sb[:],
                    in_=wq_c[c * 128:(c + 1) * 128, :].rearrange(
                        "p (cc f) -> p cc f", cc=CC),
                    cond=cond)
                nc.sync.dma_start(
                    out=b_sb[:], in_=b_c[c * 128:(c + 1) * 128, :], cond=cond)
                nc.sync.dma_start(
                    out=w2_sb[:], in_=w2_c[c * 128:(c + 1) * 128, :], cond=cond)
            nc.vector.memset(ones64[:], 1.0)
            nc.vector.memset(bias_m4[:], EXP_BIAS)

            def emit_body(rep):
                # constant 1.0 columns (per-head softmax-denominator rows)
                nc.vector.memset(v_nat[:, :, 64:65], 1.0)
                nc.vector.memset(v_nat[:, :, 129:130], 1.0)

                for h in range(NAG):
                    nc.scalar.dma_start(
                        out=xg_in_h[h][:],
                        in_=xs_d[h * HROWS:(h + 1) * HROWS, :])
                for h in range(NAG):
                    if not sim_mode:
                        nc.gpsimd.collective_compute(
                            "AllGather", mybir.AluOpType.bypass,
                            replica_groups=GROUPS,
                            ins=[xg_in_h[h][:].opt()],
                            outs=[xg_h[h][:].opt()])

                # ---- phase 1 (per batch): q^T,k^T = w^T @ x^T with bias on
                # evac; v computed in natural [token, feat] layout
                def emit_qkv(tb):
                    xt = xt_pool.tile([128, CC, 512], f16, tag="xt",
                                      name=f"xt_{rep}_{tb}")
                    for h in range(2):
                        g = h if ag_split else 0
                        r0 = tb * HROWS + (h * 128 if not ag_split else 0)
                        nc.sync.dma_start(
                            out=xt[:, h * HCC:(h + 1) * HCC, :],
                            in_=xg_h[g][r0:r0 + 128, :].rearrange(
                                "p (cc t) -> p cc t", cc=HCC))
                    xts = [xt[:, ci, :] for ci in range(CC)]
                    for fc in range(2):
                        ps = psum_s.tile([128, 512], f32, tag="s",
                                         name=f"ps1_{rep}_{tb}_{fc}")
                        for ci in range(CC):
                            nc.tensor.matmul(
                                ps[:],
                                w_sb[:, ci, fc * 128:(fc + 1) * 128],
                                xts[ci],
                                start=(ci == 0), stop=(ci == CC - 1))
                        nc.vector.tensor_scalar_add(
                            (qT if fc == 0 else kT)[:, tb * 512:(tb + 1) * 512],
                            ps[:], b_sb[:, fc:fc + 1])
                    for tcq in range(4):
                        tcg = tb * 4 + tcq
                        pv = psum_o.tile([128, 512], f32, tag="po",
                                         name=f"pv_{rep}_{tcg}")
                        for ci in range(CC):
                            nc.tensor.matmul(
                                pv[:, 0:128],
                                xt[:, ci, tcq * 128:(tcq + 1) * 128],
                                w_sb[:, ci, 256:384],
                                start=(ci == 0), stop=(ci == CC - 1))
                        # strided copy: pv cols [0:64],[64:128] land at
                        # v_nat[:, tcg, 0:64] and [65:129] (skip ones col)
                        src = pv[:, 0:128]
                        dst = v_nat[:, tcg, 0:129]
                        nc.vector.tensor_copy(
                            bass.AP(tensor=dst.tensor, offset=dst.offset,
                                    ap=[list(dst.ap[0]), [65, 2], [1, 64]]),
                            bass.AP(tensor=src.tensor, offset=src.offset,
                                    ap=[list(src.ap[0]), [64, 2], [1, 64]]))

                # ---- phase 2: attention per (batch, head) ----
                def emit_attention(b):
                    for qb in range(NQB):
                        qsl = slice(b * N + qb * 512, b * N + (qb + 1) * 512)
                        po = [psum_o.tile([128, 512], f32, tag="po",
                                          name=f"po_{rep}_{b}_{qb}_{h}")
                              for h in range(HPC)]
                        for kcg in range(NKC // 2):
                            exs = {}
                            for h in range(HPC):
                                hsl = slice(h * 64, (h + 1) * 64)
                                ps = psum_s.tile(
                                    [128, 1024], f32, tag="s",
                                    name=f"ps2_{rep}_{b}_{qb}_{kcg}_{h}")
                                for kc2 in range(2):
                                    kc = kcg * 2 + kc2
                                    ksl = slice(b * N + kc * 128,
                                                b * N + (kc + 1) * 128)
                                    nc.tensor.matmul(
                                        ps[:, kc2 * 512:(kc2 + 1) * 512],
                                        kT[hsl, ksl], qT[hsl, qsl],
                                        start=True, stop=True)
                                ex = exp_pool.tile(
                                    [128, 1024], f16, tag="ex",
                                    name=f"ex_{rep}_{b}_{qb}_{kcg}_{h}")
                                nc.scalar.activation(ex[:], ps[:], Exp,
                                                     scale=float(SCALE),
                                                     bias=bias_m4[:])
                                exs[h] = ex
                            for kc2 in range(2):
                                kc = kcg * 2 + kc2
                                tcg = b * NKC + kc
                                for h in range(HPC):
                                    nc.tensor.matmul(
                                        po[h][0:65, :],
                                        v_nat[:, tcg, h * 65:(h + 1) * 65],
                                        exs[h][:, kc2 * 512:(kc2 + 1) * 512],
                                        start=(kc == 0),
                                        stop=(kc == NKC - 1))
                        for h in range(HPC):
                            # broadcast denom row across partitions via a PE
                            # outer product, then reciprocal + multiply on DVE
                            s_sb = small_pool.tile(
                                [1, 512], f16, tag="r",
                                name=f"s_sb_{rep}_{b}_{qb}_{h}")
                            nc.vector.tensor_copy(s_sb[:], po[h][64:65, :])
                            pr = psum_aux.tile([64, 512], f32, tag="aux",
                                               name=f"pr_{rep}_{b}_{qb}_{h}")
                            nc.tensor.matmul(pr[:], ones64[:], s_sb[:],
                                             start=True, stop=True)
                            rcp = small_pool.tile(
                                [64, 512], f32, tag="rb",
                                name=f"rcp_{rep}_{b}_{qb}_{h}")
                            nc.vector.reciprocal(rcp[:], pr[:])
                            nc.vector.tensor_tensor(
                                ohT[h * 64:(h + 1) * 64, qsl],
                                po[h][0:64, :], rcp[:], Mult)

                        # ---- phase 3 interleaved: project this q-block's
                        # 4 token chunks while the next q-block computes ----
                        for tcq in range(4):
                            tcg = b * 16 + qb * 4 + tcq
                            pp = psum_aux.tile([128, 1024], f32, tag="aux",
                                               name=f"pp_{rep}_{tcg}")
                            for jh in range(C // 512):
                                nc.tensor.matmul(
                                    pp[:, jh * 512:(jh + 1) * 512],
                                    ohT[:, tcg * 128:(tcg + 1) * 128],
                                    w2_sb[:, jh * 512:(jh + 1) * 512],
                                    start=True, stop=True)
                            ob = out_pool.tile([128, 1024], f16, tag="ob",
                                               name=f"ob_{rep}_{tcg}")
                            nc.vector.tensor_copy(ob[:], pp[:])
                            rsb = b if rs_split else 0
                            lr = tcg * 128 - rsb * N  # row within op_b[rsb]
                            nc.sync.dma_start(
                                out=op_b[rsb][lr:lr + 128, :], in_=ob[:])

                for b in range(B):
                    for tb in range(b * NTB // B, (b + 1) * NTB // B):
                        emit_qkv(tb)
                    emit_attention(b)
                    # batch b's reduce-scatter overlaps batch b+1's compute
                    if rs_split:
                        if not sim_mode:
                            nc.gpsimd.collective_compute(
                                "ReduceScatter", mybir.AluOpType.add,
                                replica_groups=GROUPS,
                                ins=[op_b[b][:].opt()],
                                outs=[os_b[b][:].opt()])
                        nc.sync.dma_start(out=out_d[b, :, :], in_=os_b[b][:])
                if not rs_split:
                    if not sim_mode:
                        nc.gpsimd.collective_compute(
                            "ReduceScatter", mybir.AluOpType.add,
                            replica_groups=GROUPS,
                            ins=[op_b[0][:].opt()], outs=[os_b[0][:].opt()])
                    nc.sync.dma_start(out=out_d[:].opt(),
                                      in_=os_b[0][:].opt())

            for rep in range(reps):
                emit_body(rep)

    nc.compile()
    return nc


def _weights_key(w_qkv, b_qkv, w_proj):
    import hashlib
    h = hashlib.sha1()
    for a in (w_qkv, b_qkv, w_proj):
        h.update(np.ascontiguousarray(a, dtype=np.float32).tobytes())
    return h.hexdigest()


def get_program(w_qkv, b_qkv, w_proj):
    key = _weights_key(w_qkv, b_qkv, w_proj)
    if _CACHE.get("key") != key:
        _CACHE["nc"] = _build_program(w_qkv, b_qkv, w_proj)
        _CACHE["key"] = key
    return _CACHE["nc"]


def build_null_program():
    """Tiny kernel for calibrating per-dispatch overhead in test harnesses."""
    import concourse.mybir as mybir
    import concourse.tile as tile
    from concourse import bacc

    f32 = mybir.dt.float32
    nc = bacc.Bacc("TRN2", target_bir_lowering=False, debug=False,
                   num_devices=NCORES)
    x_in = nc.dram_tensor("x", [128, 128], f32, kind="ExternalInput")
    y_out = nc.dram_tensor("y", [128, 128], f32, kind="ExternalOutput")
    with tile.TileContext(nc) as tc:
        with tc.tile_pool(name="p", bufs=1) as pool:
            t = pool.tile([128, 128], f32)
            nc.sync.dma_start(out=t[:], in_=x_in[:])
            nc.sync.dma_start(out=y_out[:], in_=t[:])
    nc.compile()
    x = np.zeros((128, 128), dtype=np.float32)
    return nc, [{"x": x} for _ in range(NCORES)]


def make_in_maps(x, w_qkv=None, b_qkv=None, w_proj=None):
    """Host-side sharding: per-core input dicts (fp16 x-shard only),
    partition-major per channel-half to match the device layout."""
    HCC = C // 256
    xT = np.ascontiguousarray(x.reshape(T, C).T).astype(np.float16)
    maps = []
    for core in range(NCORES):
        xs = xT[:, core * SHARD:(core + 1) * SHARD]
        xs = xs.reshape(2, HCC, 128, SHARD).transpose(0, 2, 1, 3)
        maps.append({"xs": np.ascontiguousarray(
            xs.reshape(256, HCC * SHARD))})
    return maps


def combine_results(results, b_qkv, w_proj, b_proj):
    """Host-side unshard: interleave the per-batch output shards, add the
    effective bias (v bias passes through softmax + projection)."""
    b_eff = (b_proj.astype(np.float64)
             + b_qkv[2 * C:].astype(np.float64) @ w_proj.astype(np.float64))
    acc = np.empty((B, N, C), np.float32)
    for c, res in enumerate(results):
        sh = np.asarray(res["out_sh"]).astype(np.float32)
        for b in range(B):
            acc[b, c * OSH:(c + 1) * OSH] = sh[b]
    return acc + b_eff.astype(np.float32)[None, None, :]


def kernel(x, w_qkv, b_qkv, w_proj, b_proj):
    from concourse.bass_utils import run_bass_kernel_spmd

    x = np.asarray(x, dtype=np.float32)
    w_qkv = np.asarray(w_qkv, dtype=np.float32)
    b_qkv = np.asarray(b_qkv, dtype=np.float32)
    w_proj = np.asarray(w_proj, dtype=np.float32)
    b_proj = np.asarray(b_proj, dtype=np.float32)

    nc = get_program(w_qkv, b_qkv, w_proj)
    in_maps = make_in_maps(x)
    res = run_bass_kernel_spmd(nc, in_maps, list(range(NCORES)))
    return combine_results(res.results, b_qkv, w_proj, b_proj)


# revision 50
# speedup vs baseline: 1924.3260x; 418.9170x over previous
"""Multi-head self-attention Trainium2 kernel (8-core head-parallel, v3).

Problem: B=2, N=2048, C=1024, H=16 heads, HD=64.

The graded wall-time is dominated by host<->device I/O shipping (the axon
tunnel moves every NEFF ExternalInput/Output on each call at ~0.5 ms/MB/
core), so this version minimizes per-call tunnel bytes:

  * weights are baked into the NEFF as inline consts (loaded to device
    DRAM once at model-load time, not per call); each core picks its
    per-head slice with partition_id-predicated DMAs.
  * input: each core receives only ITS token shard of x (x^T slice
    [C, 512] in fp16, 1 MB); the full x^T is reassembled on-device with
    an AllGather over NeuronLink.
  * output: the 8 partial output projections are summed on-device with
    per-batch ReduceScatter(add) (batch 0's reduce overlaps batch 1's
    compute), so each core ships back only 512 rows of the final
    [4096, 1024] output in fp16 (1 MB).

Compute (per core, 2 heads): all matmuls in fp16 (double PE rate):
  1. qkv: q^T,k^T = w^T @ x^T per 512-token block (contraction over C in
     8 chunks), q/k bias added during PSUM->SBUF evacuation.  v is
     produced directly in natural [token, feat] layout (x-chunk
     stationary, w_v moving) so no PE transpose is needed; a constant
     1.0 column per head is memset so attn@v also yields the softmax
     denominators.  The v bias is folded into the host-side output bias
     (softmax rows sum to 1, so it passes through as b_v @ w_proj).
  2. attention per (batch, head): score chunks on PE, exp(s/8 - 4) on
     ACT straight out of PSUM (the -4 bias cancels in normalization and
     keeps fp16 exp comfortably in range), attn@v accumulated over 16
     key chunks into PSUM [65, 512] (row 64 = denominators).
  3. normalization: denominators broadcast across partitions via a PE
     outer product, reciprocal + multiply on DVE -> oh^T fp16.
  4. partial projection oh^T @ w2 -> DRAM bounce, then ReduceScatter.
"""

import numpy as np

B, N, C = 2, 2048, 1024
H = 16
HD = C // H  # 64
SCALE = HD ** -0.5
T = B * N  # 4096 tokens
NCORES = 8
HPC = H // NCORES  # 2 heads per core
SHARD = T // NCORES  # 512 tokens per core
OSH = N // NCORES  # 256 output rows per core per batch
EXP_BIAS = -4.0

_CACHE = {}


def _prep_weights(w_qkv, b_qkv, w_proj):
    """Stack per-core weight slices for the inline-const tensors.

    wq is stored partition-major ([128, CC*384] per core) so the SBUF load
    is one fully-contiguous DMA instead of 1024 x 768B strided rows."""
    CC = C // 128
    F = 3 * HPC * HD
    wq_all = np.empty((NCORES * 128, CC * F), np.float16)
    b_all = np.empty((NCORES * 128, 2), np.float32)
    w2_all = np.empty((NCORES * HPC * HD, C), np.float16)
    for core in range(NCORES):
        heads = [core * HPC + h for h in range(HPC)]
        cols = []
        for s in range(3):  # q, k, v groups -> [qA qB kA kB vA vB]
            for h in heads:
                cols.append(np.arange(s * C + h * HD, s * C + (h + 1) * HD))
        cols = np.concatenate(cols)
        wq_core = w_qkv[:, cols].astype(np.float16)  # [C, F]
        # [C, F] -> [p, cc, F] -> [128, CC*F]
        wq_all[core * 128:(core + 1) * 128] = (
            wq_core.reshape(CC, 128, F).transpose(1, 0, 2).reshape(128, CC * F))
        b_all[core * 128:(core + 1) * 128] = (
            b_qkv[cols[:256]].reshape(2, HPC * HD).T.astype(np.float32))
        rows = np.concatenate(
            [np.arange(h * HD, (h + 1) * HD) for h in heads])
        w2_all[core * 128:(core + 1) * 128] = w_proj[rows, :].astype(np.float16)
    return wq_all, b_all, w2_all


def _build_program(w_qkv, b_qkv, w_proj, reps=1, sim_mode=False,
                   ag_split=True, rs_split=True):
    # sim_mode: skip collectives (unsupported by TimelineSim) so the compute
    # portion can be timeline-profiled single-core; numerics are garbage.
    # ag_split/rs_split: emit the x AllGather / output ReduceScatter as two
    # halves (overlap) or one collective each (less per-collective overhead).
    import concourse.bass as bass
    import concourse.mybir as mybir
    import concourse.tile as tile
    from concourse import bacc

    f16 = mybir.dt.float16
    f32 = mybir.dt.float32
    Exp = mybir.ActivationFunctionType.Exp
    Mult = mybir.AluOpType.mult

    wq_all, b_all, w2_all = _prep_weights(w_qkv, b_qkv, w_proj)

    nc = bacc.Bacc("TRN2", target_bir_lowering=False, debug=False,
                   num_devices=NCORES)

    # x shard ships partition-major per channel-half ([2*128, 4*512]) so the
    # bounce, the gathered-block reads, and the SBUF tile are all contiguous
    CC = C // 128
    HCC = CC // 2
    xs_d = nc.dram_tensor("xs", [2 * 128, HCC * SHARD], f16,
                          kind="ExternalInput")
    out_d = nc.dram_tensor("out_sh", [B, OSH, C], f16, kind="ExternalOutput")

    wq_c = nc.inline_tensor(wq_all, "wq_c")
    b_c = nc.inline_tensor(b_all, "b_c")
    w2_c = nc.inline_tensor(w2_all, "w2_c")

    # collective bounce buffers (outputs Shared for the fast HBM-HBM path).
    # The x AllGather is split by channel halves so the qkv contraction can
    # start on ci 0..3 while the second half is still gathering.
    NAG = 2 if ag_split else 1
    HROWS = 256 // NAG  # rows per gather chunk (128 per half, or all 256)
    xg_in_h = [nc.dram_tensor(f"xg_in{h}", [HROWS, HCC * SHARD], f16,
                              kind="Internal") for h in range(NAG)]
    xg_h = [nc.dram_tensor(f"xg{h}", [NCORES * HROWS, HCC * SHARD], f16,
                           kind="Internal", addr_space="Shared")
            for h in range(NAG)]
    NRS = B if rs_split else 1
    RSROWS = T // NRS
    op_b = [nc.dram_tensor(f"op{b}", [RSROWS, C], f16, kind="Internal")
            for b in range(NRS)]
    os_b = [nc.dram_tensor(f"os{b}", [RSROWS // NCORES, C], f16,
                           kind="Internal") for b in range(NRS)]

    CC = C // 128          # 8 contraction chunks
    NTB = T // 512         # 8 token blocks (= shards)
    NKC = N // 128         # 16 key chunks per batch
    NQB = N // 512         # 4 query blocks per batch
    NTC = T // 128         # 32 token chunks
    GROUPS = [list(range(NCORES))]

    pid = nc.partition_id()

    with tile.TileContext(nc) as tc:
        with tc.tile_pool(name="persist", bufs=1) as persist, \
             tc.tile_pool(name="xt", bufs=3, space="SBUF") as xt_pool, \
             tc.tile_pool(name="exp", bufs=6) as exp_pool, \
             tc.tile_pool(name="small", bufs=4) as small_pool, \
             tc.tile_pool(name="ob", bufs=3) as out_pool, \
             tc.tile_pool(name="ps", bufs=2, space="PSUM") as psum_s, \
             tc.tile_pool(name="aux", bufs=1, space="PSUM") as psum_aux, \
             tc.tile_pool(name="po", bufs=2, space="PSUM") as psum_o:

            w_sb = persist.tile([128, CC, 3 * HPC * HD], f16, tag="w_sb")
            b_sb = persist.tile([128, 2], f32, tag="b_sb")
            w2_sb = persist.tile([128, C], f16, tag="w2_sb")
            qT = persist.tile([128, T], f16, tag="qT")
            kT = persist.tile([128, T], f16, tag="kT")
            # natural-layout v, per token-chunk: [vA(64) | 1 | vB(64) | 1]
            v_nat = persist.tile([128, NTC, 130], f16, tag="v_nat")
            ohT = persist.tile([128, T], f16, tag="ohT")
            ones64 = persist.tile([1, 64], f16, tag="ones64")
            bias_m4 = persist.tile([128, 1], f32, tag="bias_m4")

            # per-core weight selection: 8 predicated DMAs, 7 skip
            # (wq const is partition-major: contiguous [128, CC*384] rows)
            for c in range(NCORES):
                cond = pid == c
                nc.sync.dma_start(
                    out=w_sb[:],
                    in_=wq_c[c * 128:(c + 1) * 128, :].rearrange(
                        "p (cc f) -> p cc f", cc=CC),
                    cond=cond)
                nc.sync.dma_start(
                    out=b_sb[:], in_=b_c[c * 128:(c + 1) * 128, :], cond=cond)
                nc.sync.dma_start(
                    out=w2_sb[:], in_=w2_c[c * 128:(c + 1) * 128, :], cond=cond)
            nc.vector.memset(ones64[:], 1.0)
            nc.vector.memset(bias_m4[:], EXP_BIAS)

            def emit_body(rep):
                # constant 1.0 columns (per-head softmax-denominator rows)
                nc.vector.memset(v_nat[:, :, 64:65], 1.0)
                nc.vector.memset(v_nat[:, :, 129:130], 1.0)

                for h in range(NAG):
                    nc.scalar.dma_start(
                        out=xg_in_h[h][:],
                        in_=xs_d[h * HROWS:(h + 1) * HROWS, :])
                for h in range(NAG):
                    if not sim_mode:
                        nc.gpsimd.collective_compute(
                            "AllGather", mybir.AluOpType.bypass,
                            replica_groups=GROUPS,
                            ins=[xg_in_h[h][:].opt()],
                            outs=[xg_h[h][:].opt()])

                # ---- phase 1 (per batch): q^T,k^T = w^T @ x^T with bias on
                # evac; v computed in natural [token, feat] layout
                def emit_qkv(tb):
                    xt = xt_pool.tile([128, CC, 512], f16, tag="xt",
                                      name=f"xt_{rep}_{tb}")
                    for h in range(2):
                        g = h if ag_split else 0
                        r0 = tb * HROWS + (h * 128 if not ag_split else 0)
                        nc.sync.dma_start(
                            out=xt[:, h * HCC:(h + 1) * HCC, :],
                            in_=xg_h[g][r0:r0 + 128, :].rearrange(
                                "p (cc t) -> p cc t", cc=HCC))
                    xts = [xt[:, ci, :] for ci in range(CC)]
                    for fc in range(2):
                        ps = psum_s.tile([128, 512], f32, tag="s",
                                         name=f"ps1_{rep}_{tb}_{fc}")
                        for ci in range(CC):
                            nc.tensor.matmul(
                                ps[:],
                                w_sb[:, ci, fc * 128:(fc + 1) * 128],
                                xts[ci],
                                start=(ci == 0), stop=(ci == CC - 1))
                        nc.vector.tensor_scalar_add(
                            (qT if fc == 0 else kT)[:, tb * 512:(tb + 1) * 512],
                            ps[:], b_sb[:, fc:fc + 1])
                    for tcq in range(4):
                        tcg = tb * 4 + tcq
                        pv = psum_o.tile([128, 512], f32, tag="po",
                                         name=f"pv_{rep}_{tcg}")
                        for ci in range(CC):
                            nc.tensor.matmul(
                                pv[:, 0:128],
                                xt[:, ci, tcq * 128:(tcq + 1) * 128],
                                w_sb[:, ci, 256:384],
                                start=(ci == 0), stop=(ci == CC - 1))
                        # strided copy: pv cols [0:64],[64:128] land at
                        # v_nat[:, tcg, 0:64] and [65:129] (skip ones col)
                        src = pv[:, 0:128]
                        dst = v_nat[:, tcg, 0:129]
                        nc.vector.tensor_copy(
                            bass.AP(tensor=dst.tensor, offset=dst.offset,
                                    ap=[list(dst.ap[0]), [65, 2], [1, 64]]),
                            bass.AP(tensor=src.tensor, offset=src.offset,
                                    ap=[list(src.ap[0]), [64, 2], [1, 64]]))

                # ---- phase 2: attention per (batch, head) ----
                def emit_attention(b):
                    for qb in range(NQB):
                        qsl = slice(b * N + qb * 512, b * N + (qb + 1) * 512)
                        po = [psum_o.tile([128, 512], f32, tag="po",
                                          name=f"po_{rep}_{b}_{qb}_{h}")
                              for h in range(HPC)]
                        for kcg in range(NKC // 2):
                            exs = {}
                            for h in range(HPC):
                                hsl = slice(h * 64, (h + 1) * 64)
                                ps = psum_s.tile(
                                    [128, 1024], f32, tag="s",
                                    name=f"ps2_{rep}_{b}_{qb}_{kcg}_{h}")
                                for kc2 in range(2):
                                    kc = kcg * 2 + kc2
                                    ksl = slice(b * N + kc * 128,
                                                b * N + (kc + 1) * 128)
                                    nc.tensor.matmul(
                                        ps[:, kc2 * 512:(kc2 + 1) * 512],
                                        kT[hsl, ksl], qT[hsl, qsl],
                                        start=True, stop=True)
                                ex = exp_pool.tile(
                                    [128, 1024], f16, tag="ex",
                                    name=f"ex_{rep}_{b}_{qb}_{kcg}_{h}")
                                nc.scalar.activation(ex[:], ps[:], Exp,
                                                     scale=float(SCALE),
                                                     bias=bias_m4[:])
                                exs[h] = ex
                            for kc2 in range(2):
                                kc = kcg * 2 + kc2
                                tcg = b * NKC + kc
                                for h in range(HPC):
                                    nc.tensor.matmul(
                                        po[h][0:65, :],
                                        v_nat[:, tcg, h * 65:(h + 1) * 65],
                                        exs[h][:, kc2 * 512:(kc2 + 1) * 512],
                                        start=(kc == 0),
                                        stop=(kc == NKC - 1))
                        for h in range(HPC):
                            # broadcast denom row across partitions via a PE
                            # outer product, then reciprocal + multiply on DVE
                            s_sb = small_pool.tile(
                                [1, 512], f16, tag="r",
                                name=f"s_sb_{rep}_{b}_{qb}_{h}")
                            nc.vector.tensor_copy(s_sb[:], po[h][64:65, :])
                            pr = psum_aux.tile([64, 512], f32, tag="aux",
                                               name=f"pr_{rep}_{b}_{qb}_{h}")
                            nc.tensor.matmul(pr[:], ones64[:], s_sb[:],
                                             start=True, stop=True)
                            rcp = small_pool.tile(
                                [64, 512], f32, tag="rb",
                                name=f"rcp_{rep}_{b}_{qb}_{h}")
                            nc.vector.reciprocal(rcp[:], pr[:])
                            nc.vector.tensor_tensor(
                                ohT[h * 64:(h + 1) * 64, qsl],
                                po[h][0:64, :], rcp[:], Mult)

                        # ---- phase 3 interleaved: project this q-block's
                        # 4 token chunks while the next q-block computes ----
                        for tcq in range(4):
                            tcg = b * 16 + qb * 4 + tcq
                            pp = psum_aux.tile([128, 1024], f32, tag="aux",
                                               name=f"pp_{rep}_{tcg}")
                            for jh in range(C // 512):
                                nc.tensor.matmul(
                                    pp[:, jh * 512:(jh + 1) * 512],
                                    ohT[:, tcg * 128:(tcg + 1) * 128],
                                    w2_sb[:, jh * 512:(jh + 1) * 512],
                                    start=True, stop=True)
                            ob = out_pool.tile([128, 1024], f16, tag="ob",
                                               name=f"ob_{rep}_{tcg}")
                            nc.vector.tensor_copy(ob[:], pp[:])
                            rsb = b if rs_split else 0
                            lr = tcg * 128 - rsb * N  # row within op_b[rsb]
                            nc.sync.dma_start(
                                out=op_b[rsb][lr:lr + 128, :], in_=ob[:])

                for b in range(B):
                    for tb in range(b * NTB // B, (b + 1) * NTB // B):
                        emit_qkv(tb)
                    emit_attention(b)
                    # batch b's reduce-scatter overlaps batch b+1's compute
                    if rs_split:
                        if not sim_mode:
                            nc.gpsimd.collective_compute(
                                "ReduceScatter", mybir.AluOpType.add,
                                replica_groups=GROUPS,
                                ins=[op_b[b][:].opt()],
                                outs=[os_b[b][:].opt()])
                        nc.sync.dma_start(out=out_d[b, :, :], in_=os_b[b][:])
                if not rs_split:
                    if not sim_mode:
                        nc.gpsimd.collective_compute(
                            "ReduceScatter", mybir.AluOpType.add,
                            replica_groups=GROUPS,
                            ins=[op_b[0][:].opt()], outs=[os_b[0][:].opt()])
                    nc.sync.dma_start(out=out_d[:].opt(),
                                      in_=os_b[0][:].opt())

            for rep in range(reps):
                emit_body(rep)

    nc.compile()
    return nc


def _weights_key(w_qkv, b_qkv, w_proj):
    import hashlib
    h = hashlib.sha1()
    for a in (w_qkv, b_qkv, w_proj):
        h.update(np.ascontiguousarray(a, dtype=np.float32).tobytes())
    return h.hexdigest()


def get_program(w_qkv, b_qkv, w_proj):
    key = _weights_key(w_qkv, b_qkv, w_proj)
    if _CACHE.get("key") != key:
        _CACHE["nc"] = _build_program(w_qkv, b_qkv, w_proj)
        _CACHE["key"] = key
    return _CACHE["nc"]


def build_null_program():
    """Tiny kernel for calibrating per-dispatch overhead in test harnesses."""
    import concourse.mybir as mybir
    import concourse.tile as tile
    from concourse import bacc

    f32 = mybir.dt.float32
    nc = bacc.Bacc("TRN2", target_bir_lowering=False, debug=False,
                   num_devices=NCORES)
    x_in = nc.dram_tensor("x", [128, 128], f32, kind="ExternalInput")
    y_out = nc.dram_tensor("y", [128, 128], f32, kind="ExternalOutput")
    with tile.TileContext(nc) as tc:
        with tc.tile_pool(name="p", bufs=1) as pool:
            t = pool.tile([128, 128], f32)
            nc.sync.dma_start(out=t[:], in_=x_in[:])
            nc.sync.dma_start(out=y_out[:], in_=t[:])
    nc.compile()
    x = np.zeros((128, 128), dtype=np.float32)
    return nc, [{"x": x} for _ in range(NCORES)]


def make_in_maps(x, w_qkv=None, b_qkv=None, w_proj=None):
    """Host-side sharding: per-core input dicts (fp16 x-shard only),
    partition-major per channel-half to match the device layout."""
    HCC = C // 256
    xT = np.ascontiguousarray(x.reshape(T, C).T).astype(np.float16)
    maps = []
    for core in range(NCORES):
        xs = xT[:, core * SHARD:(core + 1) * SHARD]
        xs = xs.reshape(2, HCC, 128, SHARD).transpose(0, 2, 1, 3)
        maps.append({"xs": np.ascontiguousarray(
            xs.reshape(256, HCC * SHARD))})
    return maps


def combine_results(results, b_qkv, w_proj, b_proj):
    """Host-side unshard: interleave the per-batch output shards, add the
    effective bias (v bias passes through softmax + projection)."""
    b_eff = (b_proj.astype(np.float64)
             + b_qkv[2 * C:].astype(np.float64) @ w_proj.astype(np.float64))
    acc = np.empty((B, N, C), np.float32)
    for c, res in enumerate(results):
        sh = np.asarray(res["out_sh"]).astype(np.float32)
        for b in range(B):
            acc[b, c * OSH:(c + 1) * OSH] = sh[b]
    return acc + b_eff.astype(np.float32)[None, None, :]


def kernel(x, w_qkv, b_qkv, w_proj, b_proj):
    from concourse.bass_utils import run_bass_kernel_spmd

    x = np.asarray(x, dtype=np.float32)
    w_qkv = np.asarray(w_qkv, dtype=np.float32)
    b_qkv = np.asarray(b_qkv, dtype=np.float32)
    w_proj = np.asarray(w_proj, dtype=np.float32)
    b_proj = np.asarray(b_proj, dtype=np.float32)

    nc = get_program(w_qkv, b_qkv, w_proj)
    in_maps = make_in_maps(x)
    res = run_bass_kernel_spmd(nc, in_maps, list(range(NCORES)))
    return combine_results(res.results, b_qkv, w_proj, b_proj)
